# revision 33
# baseline (speedup 1.0000x reference)
"""AELoss (associative-embedding push/pull loss) on 8 TRN2 NeuronCores.

Strategy: data-parallel over batch — each of the 8 cores handles 4 images.
The tags tensor is huge ([B, N, 1], N = 17*256*256) but only the visible
(person, joint) pairs are ever needed (vis masks the rest), so the host
compacts just those ~1020 indices per core into [128, C] slots (C ~ 9) and
the kernel issues ONE [128,1] indirect DMA per slot column — the per-
instruction cost of the SWDGE indirect gather is fixed (~1.45us), so
halving the column count nearly halves the gather block.

Per-person sums are recovered from the packed layout with one tiny PE
matmul per column against a host-built one-hot slot->person matrix,
accumulated in PSUM ([128,2] = [sum g, sum g^2] per person) — all hidden
under the remaining gathers. Per-image sums then go through a second
selection matmul, the pairwise push term exp(-(mi-mj)^2) uses the ACT
engine with the pair-validity mask folded in as an additive -200 penalty
(via the matmul and the activation bias), and accum_out provides the row
sums. Output [4, 2] = (push, pull) per image per core.

Identities used (exact against the reference):
  pull_pp = sum(g^2 v)/safe_cnt - mean^2
  pull    = pull_num / max(n,1)        (n>0 guard redundant: n=0 -> 0)
  push    = (S - n)/max(n^2-n, 1)*0.5  (n<2 guard redundant: S=n for n<=1)
"""

import numpy as np

B, M, K = 32, 30, 17
N = 17 * 256 * 256
NCORES = 8
BL = B // NCORES          # images per core
P = 128
PERS = BL * M             # persons per core (120)

# aux input layout (f32 columns): vis | sel | oneh | -200 bias col
C_VIS = 0
C_SEL = K                 # [17, 21)
C_ONEH = K + BL           # [21, 51)
C_BIAS = C_ONEH + M       # [51, 52)
W_AUX = C_BIAS + 1        # 52

_cache = {}


def _constants():
    p = np.arange(P)
    m = p < PERS
    sel = np.zeros((P, BL), np.float32)
    sel[p[m], (p // M)[m]] = 1.0
    selT = np.ascontiguousarray(sel.T)             # [BL, P]
    oneh = np.zeros((P, M), np.float32)
    oneh[p[m], (p % M)[m]] = 1.0
    w_img = (sel @ sel.T).astype(np.float32)       # [P, P] same-image mask
    return sel, selT, oneh, w_img


def _strip_init_barrier(nc):
    """Drop the Bass-init const-AP memsets and the all-engine barrier that
    orders them — nothing in this kernel reads the const APs (activation
    bias is passed as an explicit AP)."""
    import concourse.mybir as mybir

    bb = nc.main_func.blocks[0]
    drop = set()
    for ins in bb.instructions:
        if isinstance(ins, (mybir.InstMemset, mybir.InstDrain, mybir.InstEventSemaphore)):
            drop.add(ins.name)
    if not drop:
        return
    keep = [ins for ins in bb.instructions if ins.name not in drop]
    del bb.instructions[:]
    for ins in keep:
        bb.add_instruction(ins)


def _build(C):
    import concourse.bass as bass
    import concourse.bacc as bacc
    import concourse.mybir as mybir
    from concourse.tile import TileContext

    f32 = mybir.dt.float32
    i32 = mybir.dt.int32
    X = mybir.AxisListType.X
    op = mybir.AluOpType

    nc = bacc.Bacc(trn_type="TRN2")
    _strip_init_barrier(nc)
    tags_d = nc.dram_tensor("tags", [BL * N, 1], f32, kind="ExternalInput")
    idx_d = nc.dram_tensor("idx", [P, C], i32, kind="ExternalInput")
    amat_d = nc.dram_tensor("amat", [P, C * P], f32, kind="ExternalInput")
    aux_d = nc.dram_tensor("aux", [P, W_AUX], f32, kind="ExternalInput")
    wimg_d = nc.dram_tensor("wimg", [P, P], f32, kind="ExternalInput")
    out_d = nc.dram_tensor("out", [BL, 2], f32, kind="ExternalOutput")

    with TileContext(nc) as tc:
        with (
            tc.tile_pool(name="sb", bufs=1) as sb,
            tc.tile_pool(name="ps", bufs=1, space="PSUM") as ps,
        ):
            # idx first, on its own HWDGE path, so the gathers start ASAP
            idx_t = sb.tile([P, C], i32)
            nc.sync.dma_start(out=idx_t[:], in_=idx_d[:])

            # gathers: one [128,1] indirect DMA per packed slot column
            gg = sb.tile([P, 2 * C], f32)
            for c in range(C):
                nc.gpsimd.indirect_dma_start(
                    out=gg[:, 2 * c:2 * c + 1],
                    out_offset=None,
                    in_=tags_d[:],
                    in_offset=bass.IndirectOffsetOnAxis(ap=idx_t[:, c:c + 1], axis=0),
                    oob_is_err=False,
                    bounds_check=BL * N - 1,
                )

            # remaining small inputs on the scalar engine's HWDGE queue
            aux_t = sb.tile([P, W_AUX], f32)
            nc.scalar.dma_start(out=aux_t[:], in_=aux_d[:])
            amat_t = sb.tile([P, C * P], f32)
            nc.scalar.dma_start(out=amat_t[:], in_=amat_d[:])
            vis_t = aux_t[:, C_VIS:C_VIS + K]
            oneh_t = aux_t[:, C_ONEH:C_ONEH + M]
            bias_c = aux_t[:, C_BIAS:C_BIAS + 1]
            wimg_raw = sb.tile([P, P], f32)
            nc.scalar.dma_start(out=wimg_raw[:], in_=wimg_d[:])

            # per-person [sum g, sum g^2] via one accumulating matmul per
            # column: square each landed column, matmul against the one-hot
            # slot->person matrix (overlaps the remaining gathers)
            ps12 = ps.tile([P, 2], f32)
            for c in range(C):
                nc.vector.tensor_mul(
                    out=gg[:, 2 * c + 1:2 * c + 2],
                    in0=gg[:, 2 * c:2 * c + 1], in1=gg[:, 2 * c:2 * c + 1],
                )
                nc.tensor.matmul(
                    out=ps12[:], lhsT=amat_t[:, c * P:(c + 1) * P],
                    rhs=gg[:, 2 * c:2 * c + 2],
                    start=(c == 0), stop=(c == C - 1),
                )

            # gather-independent prep (runs in the gather window)
            wimg_t = sb.tile([P, P], f32)
            nc.vector.tensor_copy(out=wimg_t[:], in_=wimg_raw[:])
            sel_t = sb.tile([P, BL], f32)
            nc.vector.tensor_copy(out=sel_t[:], in_=aux_t[:, C_SEL:C_SEL + BL])

            cnt = sb.tile([P, 1], f32)
            nc.vector.reduce_sum(out=cnt[:], in_=vis_t, axis=X)
            sc = sb.tile([P, 1], f32)
            nc.vector.tensor_scalar_max(out=sc[:], in0=cnt[:], scalar1=1.0)
            rc = sb.tile([P, 1], f32)
            nc.vector.reciprocal(out=rc[:], in_=sc[:])
            valid = sb.tile([P, 1], f32)
            nc.vector.tensor_scalar(
                out=valid[:], in0=cnt[:], scalar1=0.5, scalar2=None, op0=op.is_gt
            )
            v200 = sb.tile([P, 1], f32)
            nc.vector.tensor_scalar_mul(out=v200[:], in0=valid[:], scalar1=-200.0)
            oneh_rc = sb.tile([P, M], f32)
            nc.vector.tensor_tensor(
                out=oneh_rc[:], in0=oneh_t,
                in1=rc[:].to_broadcast([P, M]), op=op.mult,
            )
            sel_v = sb.tile([P, BL], f32)
            nc.vector.tensor_tensor(
                out=sel_v[:], in0=sel_t[:],
                in1=valid[:].to_broadcast([P, BL]), op=op.mult,
            )

            # rhs for the per-image reduction matmul:
            # cols 0:30 mean*onehot | 30:60 -200*valid*onehot | 60 pvpp | 61 valid
            rhs1 = sb.tile([P, 2 * M + 2], f32)
            nc.vector.tensor_tensor(
                out=rhs1[:, M:2 * M], in0=oneh_t,
                in1=v200[:].to_broadcast([P, M]), op=op.mult,
            )
            nc.vector.tensor_copy(out=rhs1[:, 2 * M + 1:2 * M + 2], in_=valid[:])

            # ---- post-gather chain (DVE reads the PSUM sums directly) ----
            # rhs1 cols 0:30 = mean*onehot, fused: (oneh*rc) * sum_g
            nc.vector.tensor_tensor(
                out=rhs1[:, 0:M], in0=oneh_rc[:],
                in1=ps12[:, 0:1].to_broadcast([P, M]), op=op.mult,
            )
            # same-image broadcast in one matmul: m2 = W^T @ rhs1[:, 0:60]
            m2 = ps.tile([P, 2 * M], f32)
            nc.tensor.matmul(
                out=m2[:], lhsT=wimg_t[:], rhs=rhs1[:, 0:2 * M], start=True, stop=True
            )

            mean = sb.tile([P, 1], f32)
            nc.vector.tensor_scalar(
                out=mean[:], in0=ps12[:, 0:1], scalar1=rc[:], scalar2=None, op0=op.mult
            )
            mean2 = sb.tile([P, 1], f32)
            nc.vector.tensor_mul(out=mean2[:], in0=mean[:], in1=mean[:])
            ppraw = sb.tile([P, 1], f32)
            nc.vector.tensor_scalar(
                out=ppraw[:], in0=ps12[:, 1:2], scalar1=rc[:], scalar2=None, op0=op.mult
            )
            # pvpp = (ppraw - mean^2) * valid
            nc.vector.tensor_scalar(
                out=rhs1[:, 2 * M:2 * M + 1], in0=ppraw[:],
                scalar1=mean2[:], scalar2=valid[:],
                op0=op.subtract, op1=op.mult,
            )

            # per-image pull_num and n (side branch off the critical path)
            m1 = ps.tile([BL, 2], f32)
            nc.tensor.matmul(
                out=m1[:], lhsT=sel_t[:], rhs=rhs1[:, 2 * M:2 * M + 2],
                start=True, stop=True,
            )
            p1s = sb.tile([BL, 2], f32)
            nc.vector.tensor_copy(out=p1s[:], in_=m1[:])

            # pairwise push term; pair mask folded in additively:
            # e = exp(-(d^2 + m2_v200col) - 200) = exp(-d^2) iff v_j else ~0
            d = sb.tile([P, M], f32)
            nc.vector.tensor_tensor(
                out=d[:], in0=m2[:, 0:M],
                in1=mean[:].to_broadcast([P, M]), op=op.subtract,
            )
            d2 = sb.tile([P, M], f32)
            nc.vector.tensor_mul(out=d2[:], in0=d[:], in1=d[:])
            x = sb.tile([P, M], f32)
            nc.vector.tensor_add(out=x[:], in0=d2[:], in1=m2[:, M:2 * M])
            e = sb.tile([P, M], f32)
            rowsum = sb.tile([P, 1], f32)
            nc.scalar.activation(
                out=e[:], in_=x[:],
                func=mybir.ActivationFunctionType.Exp,
                bias=bias_c, scale=-1.0,
                accum_out=rowsum[:],
            )
            m3 = ps.tile([BL, 1], f32)
            nc.tensor.matmul(
                out=m3[:], lhsT=sel_v[:], rhs=rowsum[:], start=True, stop=True
            )

            # ---- final per-image scalars on partitions 0..3 ----
            nn = p1s[:, 1:2]
            pn = p1s[:, 0:1]
            outt = sb.tile([BL, 2], f32)

            ndm = sb.tile([BL, 2], f32)
            nc.vector.tensor_scalar_max(out=ndm[:, 0:1], in0=nn, scalar1=1.0)
            # max(n^2 - n, 1) in one two-op tensor_scalar plus a max
            nd1 = sb.tile([BL, 1], f32)
            nc.vector.tensor_scalar(
                out=nd1[:], in0=nn, scalar1=nn, scalar2=nn,
                op0=op.mult, op1=op.subtract,
            )
            nc.vector.tensor_scalar_max(out=ndm[:, 1:2], in0=nd1[:], scalar1=1.0)
            rr = sb.tile([BL, 2], f32)
            nc.vector.reciprocal(out=rr[:], in_=ndm[:])
            # pull = pull_num / max(n, 1)
            nc.vector.tensor_scalar(
                out=outt[:, 1:2], in0=pn, scalar1=rr[:, 0:1], scalar2=None,
                op0=op.mult,
            )
            smn = sb.tile([BL, 1], f32)
            nc.vector.tensor_sub(out=smn[:], in0=m3[:], in1=nn)
            # push = (S - n) / max(n^2 - n, 1) * 0.5
            nc.vector.tensor_scalar(
                out=outt[:, 0:1], in0=smn[:], scalar1=rr[:, 1:2], scalar2=0.5,
                op0=op.mult, op1=op.mult,
            )

            nc.sync.dma_start(out=out_d[:], in_=outt[:])

    nc.compile()
    return nc


def _in_maps(tags, joints):
    sel, selT, oneh, w_img = _constants()
    tags = np.ascontiguousarray(np.asarray(tags, dtype=np.float32)).reshape(B, N)
    joints = np.asarray(joints, dtype=np.int32)
    idx_all = joints[..., 0]                               # [B, M, K]
    vis_all = joints[..., 1] > 0                           # [B, M, K] bool

    # compact visible (person, joint) slots per core
    per_core = []
    C = 1
    for c in range(NCORES):
        b0 = c * BL
        persons = []
        fidx = []
        for b in range(BL):
            vb = vis_all[b0 + b]                           # [M, K]
            mm, kk = np.nonzero(vb)
            persons.append(b * M + mm)
            fidx.append(idx_all[b0 + b][mm, kk] + b * N)
        persons = np.concatenate(persons)
        fidx = np.concatenate(fidx)
        per_core.append((persons, fidx))
        C = max(C, (len(fidx) + P - 1) // P)

    in_maps = []
    for c in range(NCORES):
        b0 = c * BL
        persons, fidx = per_core[c]
        n_slots = len(fidx)
        idx_l = np.zeros((P, C), np.int32)
        amat = np.zeros((P, C * P), np.float32)
        s = np.arange(n_slots)
        sp, scol = s % P, s // P
        idx_l[sp, scol] = fidx
        amat[sp, scol * P + persons] = 1.0

        aux = np.zeros((P, W_AUX), np.float32)
        for b in range(BL):
            rows = slice(b * M, (b + 1) * M)
            aux[rows, C_VIS:C_VIS + K] = vis_all[b0 + b]
        aux[:, C_SEL:C_SEL + BL] = sel
        aux[:, C_ONEH:C_ONEH + M] = oneh
        aux[:, C_BIAS] = -200.0
        in_maps.append({
            "tags": np.ascontiguousarray(tags[b0:b0 + BL].reshape(BL * N, 1)),
            "idx": idx_l,
            "amat": amat,
            "aux": aux,
            "wimg": w_img,
        })
    return (C,), in_maps


def _run(key, in_maps, trace=False):
    from concourse import bass_utils

    if key not in _cache:
        _cache[key] = _build(*key)
    return bass_utils.run_bass_kernel_spmd(
        _cache[key], in_maps, core_ids=list(range(NCORES)), trace=trace
    )


def kernel(tags, joints):
    key, in_maps = _in_maps(tags, joints)
    res = _run(key, in_maps)
    outs = [res.results[c]["out"] for c in range(NCORES)]
    push = np.concatenate([o[:, 0] for o in outs]).astype(np.float32)
    pull = np.concatenate([o[:, 1] for o in outs]).astype(np.float32)
    return push, pull
